# revision 36
# baseline (speedup 1.0000x reference)
"""Trainium2 Bass kernel for nn_CWGDN (dense_cnn): LN -> temporal pin conv ->
dynamic depthwise conv (w/ pooled kernel-generator branch) -> gate -> temporal
pout conv + residual.

Sharding: L1 data-parallel over 16 (b,t) instances (2 per core); L2
data-parallel over pixels (1/8 of all pixels per core, all 16 instances).

All depthwise convs (6-layer pyramid + dynamic 3x3) run on the Tensor engine
as diagonal-stationary matmuls accumulating in PSUM; image borders are
handled with restricted row/col rectangles (center tap first, start=True).
LayerNorm is folded into the pin matmul. The front (stats -> rsqrt ->
r-broadcast -> x*r -> pin) is chunk-interleaved to keep the PE busy.
"""
import sys

sys.path.insert(0, "/opt/trn_rl_repo")

import numpy as np
import ml_dtypes

import concourse.bass as bass
import concourse.tile as tile
from concourse import bacc, mybir
from concourse.bass_utils import run_bass_kernel_spmd

BF = ml_dtypes.bfloat16
F32 = mybir.dt.float32
BF16 = mybir.dt.bfloat16
AL = mybir.AluOpType
ACTF = mybir.ActivationFunctionType

B, T, C, H, W = 2, 8, 64, 128, 128
HID = 128
S = H * W  # 16384
K = 3
EPS = 1e-5

_cache = {}
TRACE = False
PROF = {}


def _rects(dy, dx, hh, ww):
    r0, r1 = max(0, -dy), min(hh, hh - dy)
    c0, c1 = max(0, -dx), min(ww, ww - dx)
    return r0, r1, c0, c1


def _pe_dwconv(nc, ps_pool, src_v, hh, ww, diag_aps, bias_ap, ones_ap,
               consume, band_rows):
    """Depthwise 3x3 conv + bias on TensorE via diagonal matmuls (clipped
    rectangles at image borders; center tap first with start=True)."""
    n_bands = hh // band_rows
    rg = max(1, 512 // ww)
    for b in range(n_bands):
        r0 = b * band_rows
        ps = ps_pool.tile([128, band_rows * ww], F32, tag="dw", bufs=3)
        psv = ps[:].rearrange("p (h w) -> p h w", h=band_rows)
        for r in range(0, band_rows, rg):
            re = min(band_rows, r + rg)
            nc.tensor.matmul(psv[:, r:re, :], diag_aps[4],
                             src_v[:, r0 + r : r0 + re, :],
                             start=True, stop=False)
        if bias_ap is not None:
            for r in range(0, band_rows, rg):
                re = min(band_rows, r + rg)
                nc.tensor.matmul(psv[:, r:re, :], bias_ap,
                                 ones_ap[:, : (re - r) * ww],
                                 start=False, stop=False)
        for ky in range(3):
            for kx in range(3):
                if ky == 1 and kx == 1:
                    continue
                dy, dx = ky - 1, kx - 1
                fr0, fr1, c0, c1 = _rects(dy, dx, hh, ww)
                br0, br1 = max(fr0, r0), min(fr1, r0 + band_rows)
                if br0 >= br1 or c0 >= c1:
                    continue
                last = ky == 2 and kx == 2
                cw = c1 - c0
                rg2 = max(1, 512 // cw)
                for r in range(br0, br1, rg2):
                    re = min(br1, r + rg2)
                    nc.tensor.matmul(
                        psv[:, r - r0 : re - r0, c0:c1],
                        diag_aps[ky * 3 + kx],
                        src_v[:, r + dy : re + dy, c0 + dx : c1 + dx],
                        start=False, stop=last and (re == br1))
        consume(ps, b, r0, band_rows)


def _build_l1():
    nc = bacc.Bacc("TRN2", target_bir_lowering=False, debug=False, num_devices=8)
    xh = nc.dram_tensor("xh", [4, C, S], BF16, kind="ExternalInput")
    v4b = nc.dram_tensor("v4b", [128, 4], F32, kind="ExternalInput")
    i2s = nc.dram_tensor("i2s", [128, 16 * 32], BF16, kind="ExternalInput")
    w1p = nc.dram_tensor("w1p", [2, 2, 128, 128], BF16, kind="ExternalInput")
    w1lo = nc.dram_tensor("w1lo", [2, 2, 70, 128], BF16, kind="ExternalInput")
    dstat = nc.dram_tensor("dstat", [128, 60 * 128], BF16, kind="ExternalInput")
    tokw = nc.dram_tensor("tokw", [9, 128, 128], F32, kind="ExternalInput")
    tokb = nc.dram_tensor("tokb", [128, 9], F32, kind="ExternalInput")
    dwb = nc.dram_tensor("dwb", [128, 1], F32, kind="ExternalInput")
    idm = nc.dram_tensor("idm", [128, 128], BF16, kind="ExternalInput")
    g_out = [nc.dram_tensor(f"g{j}", [128, S], BF16, kind="ExternalOutput")
             for j in range(2)]
    scr_s = nc.dram_tensor("scr_s", [2, 32, 1024], F32)
    scr_q = nc.dram_tensor("scr_q", [2, 32, 1024], F32)
    r_scr = nc.dram_tensor("r_scr", [4, S], BF16)
    mur_scr = nc.dram_tensor("mur_scr", [4, S], BF16)
    v_scr = nc.dram_tensor("v_scr", [4, S], BF16)
    x2d = nc.dram_tensor("x2d", [2, 128, S], BF16)

    with tile.TileContext(nc, pool_alloc_mode="queue") as tc:
        with tc.tile_pool(name="wp", bufs=1) as wp:
            i2s_sb = wp.tile([128, 16 * 32], BF16, tag="i2s")
            nc.scalar.dma_start(i2s_sb[:], i2s[:])
            w1p_sb, w1lo_sb = [], []
            for j in range(2):
                w1p_sb.append([])
                w1lo_sb.append([])
                for oh in range(2):
                    tp = wp.tile([128, 128], BF16, tag=f"w1p{j}{oh}")
                    nc.scalar.dma_start(tp[:], w1p[j, oh])
                    w1p_sb[j].append(tp)
                    tl = wp.tile([70, 128], BF16, tag=f"w1lo{j}{oh}")
                    nc.scalar.dma_start(tl[:], w1lo[j, oh])
                    w1lo_sb[j].append(tl)
            ds_sb = wp.tile([128, 60 * 128], BF16, tag="dstat")
            nc.scalar.dma_start(ds_sb[:], dstat[:])

            def dsa(l, k):
                n = l * 10 + k
                return ds_sb[:, n * 128 : (n + 1) * 128]

            tokw_sb = []
            for k in range(9):
                tk = wp.tile([128, 128], F32, tag=f"tokw{k}")
                nc.scalar.dma_start(tk[:], tokw[k])
                tokw_sb.append(tk)
            tokb_sb = wp.tile([128, 9], F32, tag="tokb")
            nc.scalar.dma_start(tokb_sb[:], tokb[:])
            dwb_sb = wp.tile([128, 1], F32, tag="dwb")
            nc.scalar.dma_start(dwb_sb[:], dwb[:])
            v4_sb = wp.tile([128, 4], F32, tag="v4")
            nc.scalar.dma_start(v4_sb[:], v4b[:])
            idm_sb = wp.tile([128, 128], BF16, tag="idm")
            nc.scalar.dma_start(idm_sb[:], idm[:])
            ones = wp.tile([128, 1024], BF16, tag="ones")
            nc.gpsimd.memset(ones[:, :], 1.0)
            eps_t = wp.tile([128, 1], F32, tag="eps")
            nc.gpsimd.memset(eps_t[:, :], EPS)

            with tc.tile_pool(name="cp0", bufs=1) as cp0:
                x1t = [cp0.tile([128, S], BF16, tag=f"x1_{j}", name=f"x1t{j}")
                       for j in range(2)]
                pairs = []
                with tc.tile_pool(name="pp", bufs=1) as pp:
                    # ---- stats phase ----
                    with tc.tile_pool(name="ap", bufs=1) as ap, \
                         tc.tile_pool(name="ps_a", bufs=1, space="PSUM") as psa:
                        for p in range(2):
                            xs = pp.tile([128, S], BF16, tag=f"pair{p}")
                            pairs.append(xs)
                            for h4 in range(4):
                                nc.sync.dma_start(
                                    xs[:, h4 * 4096 : (h4 + 1) * 4096],
                                    xh[2 * p : 2 * p + 2, :,
                                       h4 * 4096 : (h4 + 1) * 4096].rearrange(
                                        "s c f -> (s c) f"))
                            psS = psa.tile([32, 1024], F32, tag="stS")
                            psQ = psa.tile([32, 1024], F32, tag="stQ")
                            for qq in range(16):
                                n0 = qq * 1024
                                sq = ap.tile([128, 1024], BF16, tag="sq", bufs=3)
                                nc.scalar.activation(sq[:], xs[:, n0 : n0 + 1024],
                                                     ACTF.Square)
                                i2q = i2s_sb[:, qq * 32 : (qq + 1) * 32]
                                for hf in range(2):
                                    hs = slice(hf * 512, (hf + 1) * 512)
                                    nc.tensor.matmul(
                                        psS[:, hs], i2q,
                                        xs[:, n0 + hf * 512 : n0 + (hf + 1) * 512],
                                        start=(qq == 0), stop=(qq == 15 and hf == 1))
                                    nc.tensor.matmul(
                                        psQ[:, hs], i2q, sq[:, hs],
                                        start=(qq == 0), stop=(qq == 15 and hf == 1))
                            stS = ap.tile([32, 1024], F32, tag="stgS")
                            nc.vector.tensor_copy(stS[:], psS[:])
                            stQ = ap.tile([32, 1024], F32, tag="stgQ")
                            nc.vector.tensor_copy(stQ[:], psQ[:])
                            nc.gpsimd.dma_start(scr_s[p], stS[:])
                            nc.gpsimd.dma_start(scr_q[p], stQ[:])
                            # gather back pixel-spread (128, 256)
                            sp_s = ap.tile([128, 256], F32, tag="sp_s")
                            sp_q = ap.tile([128, 256], F32, tag="sp_q")
                            for sp_t, scr in ((sp_s, scr_s), (sp_q, scr_q)):
                                for qq in range(16):
                                    dst = sp_t[qq * 8 : (qq + 1) * 8, :].rearrange(
                                        "q (g f) -> q g f", g=2)
                                    src = scr[p, 2 * qq : 2 * qq + 2].rearrange(
                                        "g (s f) -> s g f", f=128)
                                    eng = nc.sync if qq % 2 == 0 else nc.gpsimd
                                    eng.dma_start(dst, src)
                            mu = ap.tile([128, 256], F32, tag="mu")
                            nc.scalar.mul(mu[:], sp_s[:], 1.0 / 64.0)
                            msq = ap.tile([128, 256], F32, tag="msq")
                            nc.vector.tensor_tensor(msq[:], mu[:], mu[:], op=AL.mult)
                            nc.vector.scalar_tensor_tensor(
                                sp_q[:], sp_q[:], 1.0 / 64.0, msq[:],
                                op0=AL.mult, op1=AL.subtract)
                            nc.scalar.activation(sp_s[:], sp_q[:], ACTF.Sqrt,
                                                 bias=eps_t[:, :])
                            nc.vector.reciprocal(sp_q[:], sp_s[:])
                            for g in range(2):
                                nc.vector.tensor_scalar(
                                    sp_q[:, g * 128 : (g + 1) * 128],
                                    sp_q[:, g * 128 : (g + 1) * 128],
                                    v4_sb[:, 2 * p + g : 2 * p + g + 1], None,
                                    op0=AL.mult)
                            r16 = ap.tile([128, 256], BF16, tag="r16")
                            nc.vector.tensor_copy(r16[:], sp_q[:])
                            nc.vector.tensor_tensor(mu[:], mu[:], sp_q[:],
                                                    op=AL.mult)
                            mur16 = ap.tile([128, 256], BF16, tag="mur16")
                            nc.vector.tensor_copy(mur16[:], mu[:])
                            vr16 = ap.tile([128, 256], BF16, tag="vr16")
                            nc.gpsimd.memset(vr16[:, :], 1.0)
                            for g in range(2):
                                nc.vector.tensor_scalar(
                                    vr16[:, g * 128 : (g + 1) * 128],
                                    vr16[:, g * 128 : (g + 1) * 128],
                                    v4_sb[:, 2 * p + g : 2 * p + g + 1], None,
                                    op0=AL.mult)
                            for scr2, t16 in ((r_scr, r16), (mur_scr, mur16),
                                              (v_scr, vr16)):
                                nc.scalar.dma_start(
                                    scr2[2 * p : 2 * p + 2].rearrange(
                                        "g (q f) -> q g f", q=128), t16[:])

                    # ---- pin phase: chunk-interleaved scale + matmul ----
                    with tc.tile_pool(name="ps_b", bufs=2, space="PSUM") as psb, \
                         tc.tile_pool(name="pst", bufs=1) as pst:
                        scaled = [False] * 8
                        for j in range(2):
                            for c8 in range(8):
                                n0 = c8 * 2048
                                cs = slice(n0, n0 + 2048)
                                if not scaled[c8]:
                                    scaled[c8] = True
                                    for p in range(2):
                                        rb = pst.tile([128, 2048], BF16,
                                                      tag="rb", bufs=2)
                                        for g in range(2):
                                            nc.gpsimd.dma_start(
                                                rb[g * 64 : (g + 1) * 64, :],
                                                r_scr[2 * p + g : 2 * p + g + 1,
                                                      cs].broadcast_to((64, 2048)))
                                        nc.vector.tensor_tensor(
                                            pairs[p][:, cs], pairs[p][:, cs],
                                            rb[:], op=AL.mult)
                                lot = pst.tile([70, 2048], BF16, tag="lo",
                                               bufs=4, name="lot")
                                nc.sync.dma_start(lot[64:67, :],
                                                  mur_scr[j : j + 3, cs])
                                nc.sync.dma_start(lot[67:70, :],
                                                  v_scr[j : j + 3, cs])
                                if j == 0:
                                    nc.sync.dma_start(lot[0:64, :],
                                                      pairs[1][0:64, cs])
                                else:
                                    nc.sync.dma_start(lot[0:64, :],
                                                      pairs[0][64:128, cs])
                                for oh in range(2):
                                    ps = psb.tile([128, 2048], F32, tag="piny")
                                    for c in range(4):
                                        m0 = n0 + c * 512
                                        ms = slice(c * 512, (c + 1) * 512)
                                        nc.tensor.matmul(ps[:, ms], w1p_sb[j][oh],
                                                         pairs[j][:, m0 : m0 + 512],
                                                         start=True, stop=False)
                                        nc.tensor.matmul(ps[:, ms], w1lo_sb[j][oh],
                                                         lot[:, ms],
                                                         start=False, stop=True)
                                    if oh == 0:
                                        nc.scalar.copy(x1t[j][:, cs], ps[:])
                                    else:
                                        stg2 = pst.tile([128, 2048], BF16,
                                                        tag="stg2", bufs=2)
                                        nc.scalar.copy(stg2[:], ps[:])
                                        nc.sync.dma_start(x2d[j][:, cs], stg2[:])

                # ---- conv/pool/dyn/gate phase ----
                with tc.tile_pool(name="cp1", bufs=1) as cp1, \
                     tc.tile_pool(name="ps_p", bufs=1, space="PSUM") as psp:
                    aw = [cp1.tile([128, 8192], BF16, tag=f"aw{j}", name=f"aw{j}")
                          for j in range(2)]
                    pab = [[cp1.tile([128, 4096], BF16, tag=f"p{j}{i}", name=f"p{j}{i}")
                            for i in range(2)] for j in range(2)]
                    pcd = [[cp1.tile([128, 1024], BF16, tag=f"q{j}{i}", name=f"q{j}{i}")
                            for i in range(2)] for j in range(2)]
                    pooled = cp1.tile([128, 2], F32, tag="pooled")
                    kern = [cp1.tile([128, 9], F32, tag=f"kern{j}", name=f"kern{j}")
                            for j in range(2)]

                    def mkc(d, ww):
                        def consume(ps, bi, r0, br):
                            nc.scalar.copy(d[:, r0 * ww : (r0 + br) * ww], ps[:])
                        return consume

                    x1v = [x1t[j][:].rearrange("p (h w) -> p h w", h=128)
                           for j in range(2)]
                    for j in range(2):
                        awv = aw[j][:].rearrange("p (h w) -> p h w", h=128)
                        xe = x1v[j].rearrange("p h (w2 two) -> p h w2 two", two=2)
                        nc.vector.tensor_tensor(
                            awv[:], xe[:, :, :, 0], xe[:, :, :, 1], op=AL.add)
                        pav = pab[j][0][:].rearrange("p (h w) -> p h w", h=64)
                        ae = awv.rearrange("p (h2 two) w -> p h2 two w", two=2)
                        nc.vector.tensor_tensor(
                            pav[:], ae[:, :, 0, :], ae[:, :, 1, :], op=AL.add)
                    # pyramid 64x64, layers interleaved across j
                    for l in range(3):
                        for j in range(2):
                            cur, nxt = pab[j][l % 2], pab[j][1 - l % 2]
                            _pe_dwconv(
                                nc, psp,
                                cur[:].rearrange("p (h w) -> p h w", h=64),
                                64, 64, [dsa(l, k) for k in range(9)],
                                dsa(l, 9), ones[:], mkc(nxt, 64), 16)
                            if l == 2:
                                cv = nxt[:].rearrange("p (h w) -> p h w", h=64)
                                ce = cv.rearrange(
                                    "p h (w2 two) -> p h w2 two", two=2)
                                tmv = aw[j][:, 0:2048].rearrange(
                                    "p (h w) -> p h w", h=64)
                                nc.vector.tensor_tensor(
                                    tmv[:], ce[:, :, :, 0], ce[:, :, :, 1],
                                    op=AL.max)
                                te = tmv.rearrange(
                                    "p (h2 two) w -> p h2 two w", two=2)
                                pcv = pcd[j][0][:].rearrange(
                                    "p (h w) -> p h w", h=32)
                                nc.vector.tensor_tensor(
                                    pcv[:], te[:, :, 0, :], te[:, :, 1, :],
                                    op=AL.max)
                    # pyramid 32x32
                    for l in range(3, 6):
                        for j in range(2):
                            cur, nxt = pcd[j][(l - 3) % 2], pcd[j][1 - (l - 3) % 2]
                            _pe_dwconv(
                                nc, psp,
                                cur[:].rearrange("p (h w) -> p h w", h=32),
                                32, 32, [dsa(l, k) for k in range(9)],
                                dsa(l, 9), ones[:], mkc(nxt, 32), 32)
                            if l == 5:
                                nc.scalar.activation(
                                    aw[j][:, 0:1024], nxt[:], ACTF.Copy,
                                    scale=1.0 / 1024.0,
                                    accum_out=pooled[:, j : j + 1])
                                psk = psp.tile([128, 9], F32, tag="psk", bufs=2)
                                for k in range(9):
                                    nc.tensor.matmul(psk[:, k : k + 1],
                                                     tokw_sb[k],
                                                     pooled[:, j : j + 1],
                                                     start=True, stop=True)
                                nc.vector.tensor_tensor(kern[j][:], psk[:],
                                                        tokb_sb[:], op=AL.add)
                    # dynamic dw conv + fused bias/gate
                    for j in range(2):
                        dgs = []
                        for k in range(9):
                            dg = cp1.tile([128, 128], BF16, tag=f"dg{k}", bufs=2)
                            nc.vector.tensor_scalar(
                                dg[:], idm_sb[:], kern[j][:, k : k + 1], None,
                                op0=AL.mult)
                            dgs.append(dg)

                        def consume_dyn(ps, bi, r0, br, j=j):
                            cs = slice(r0 * 128, (r0 + br) * 128)
                            x2t = cp1.tile([128, 1024], BF16, tag="x2c", bufs=3)
                            nc.gpsimd.dma_start(x2t[:], x2d[j][:, cs])
                            gst = cp1.tile([128, 1024], BF16, tag="gst", bufs=3)
                            nc.vector.scalar_tensor_tensor(
                                gst[:], ps[:], dwb_sb[:, :], x2t[:],
                                op0=AL.add, op1=AL.mult)
                            nc.sync.dma_start(g_out[j][:, cs], gst[:])

                        _pe_dwconv(nc, psp, x1v[j], 128, 128,
                                   [dg[:] for dg in dgs], None,
                                   ones[:], consume_dyn, 8)
    nc.compile()
    return nc


def _build_l2():
    """Pixel-sharded pout conv: each core handles all 16 (b,t) instances on a
    1/8 slice of the pixels (gh holds every t, so no halo duplication)."""
    nc = bacc.Bacc("TRN2", target_bir_lowering=False, debug=False, num_devices=8)
    P = 2048  # pixels per core
    gh = nc.dram_tensor("gh", [16, 128, P], BF16, kind="ExternalInput")
    xres = nc.dram_tensor("xres", [16, 64, P], BF16, kind="ExternalInput")
    w2 = nc.dram_tensor("w2", [3, 128, 64], BF16, kind="ExternalInput")
    z_out = nc.dram_tensor("z", [16, 64, P], F32, kind="ExternalOutput")
    with tile.TileContext(nc, pool_alloc_mode="queue") as tc:
        with tc.tile_pool(name="wp", bufs=1) as wp, \
             tc.tile_pool(name="ps", bufs=2, space="PSUM") as psp:
            w2_sb = []
            for tau in range(3):
                tw2 = wp.tile([128, 64], BF16, tag=f"w2{tau}")
                nc.scalar.dma_start(tw2[:], w2[tau])
                w2_sb.append(tw2)
            gsb = []
            for bt in range(16):
                g = wp.tile([128, P], BF16, tag=f"g{bt}")
                eng = nc.sync if bt % 2 == 0 else nc.scalar
                eng.dma_start(g[:], gh[bt])
                gsb.append(g)
            for bt in range(16):
                b, t = bt // 8, bt % 8
                taus = [tau for tau in range(3) if 0 <= t - 1 + tau < 8]
                xrt = wp.tile([64, P], BF16, tag="xr", bufs=3)
                nc.gpsimd.dma_start(xrt[:], xres[bt])
                for hf in range(2):
                    h0 = hf * 1024
                    ps = psp.tile([64, 1024], F32, tag="z", bufs=4)
                    for c in range(2):
                        cl = slice(c * 512, (c + 1) * 512)
                        cg = slice(h0 + c * 512, h0 + (c + 1) * 512)
                        for i, tau in enumerate(taus):
                            nc.tensor.matmul(ps[:, cl], w2_sb[tau],
                                             gsb[b * 8 + t - 1 + tau][:, cg],
                                             start=(i == 0),
                                             stop=(i == len(taus) - 1))
                    ot = wp.tile([64, 1024], F32, tag="ot", bufs=4)
                    nc.vector.tensor_tensor(ot[:], ps[:],
                                            xrt[:, h0 : h0 + 1024], op=AL.add)
                    oeng = nc.sync if hf == 0 else nc.gpsimd
                    oeng.dma_start(z_out[bt][:, h0 : h0 + 1024], ot[:])
    nc.compile()
    return nc


def _prep_weights(ln_w, ln_b, pin_w, pout_w, b1_w, b1_b, b2_w, b2_b, tok_w,
                  tok_b, dw_bias):
    pw = np.asarray(pin_w)[:, :, :, 0, 0].astype(np.float64)  # (256, 64, 3)
    lnw = np.asarray(ln_w).astype(np.float64)
    lnb = np.asarray(ln_b).astype(np.float64)
    W1 = [(pw[:, :, t] * lnw[None, :]).T for t in range(3)]
    s1 = [(pw[:, :, t] * lnw[None, :]).sum(1) for t in range(3)]
    bias1 = [pw[:, :, t] @ lnb for t in range(3)]
    w1p = np.zeros((2, 2, 128, 128), np.float32)
    w1lo = np.zeros((2, 2, 70, 128), np.float32)
    for j in range(2):
        tA, tB = (0, 1) if j == 0 else (1, 2)
        tlo = 2 if j == 0 else 0
        for oh in range(2):
            ohs = slice(oh * 128, (oh + 1) * 128)
            w1p[j, oh, 0:64] = W1[tA][:, ohs]
            w1p[j, oh, 64:128] = W1[tB][:, ohs]
            w1lo[j, oh, 0:64] = W1[tlo][:, ohs]
            for t in range(3):
                w1lo[j, oh, 64 + t] = -s1[t][ohs]
                w1lo[j, oh, 67 + t] = bias1[t][ohs]
    dstat = np.zeros((128, 60 * 128), np.float32)
    b1w = np.asarray(b1_w)[:, :, 0]
    b2w = np.asarray(b2_w)[:, :, 0]
    di = np.arange(128)
    for l in range(6):
        wl = (b1w[l] if l < 3 else b2w[l - 3]).reshape(128, 9).copy()
        if l == 0:
            wl *= 0.25
        bl = np.asarray(b1_b)[l] if l < 3 else np.asarray(b2_b)[l - 3]
        for k in range(9):
            dstat[di, (l * 10 + k) * 128 + di] = wl[:, k]
        dstat[di, (l * 10 + 9) * 128 + di] = bl
    tokw = np.zeros((9, 128, 128), np.float32)
    tw = np.asarray(tok_w)
    for k in range(9):
        tokw[k] = tw[k::9, :].T
    tokb = np.asarray(tok_b).reshape(128, 9).astype(np.float32)
    w2 = np.zeros((3, 128, 64), np.float32)
    pow_ = np.asarray(pout_w)[:, :, :, 0, 0]
    for t in range(3):
        w2[t] = pow_[:, :, t].T
    dwb = np.asarray(dw_bias).reshape(128, 1).astype(np.float32)
    idm = np.zeros((128, 128), np.float32)
    idm[di, di] = 1.0
    i2s = np.zeros((128, 16 * 32), np.float32)
    for k in range(128):
        for qq in range(16):
            i2s[k, qq * 32 + 2 * qq + k // 64] = 1.0
    return (w1p.astype(BF), w1lo.astype(BF), dstat.astype(BF), tokw, tokb,
            w2.astype(BF), dwb, idm.astype(BF), i2s.astype(BF))


def kernel(x, ln_w, ln_b, pin_w, pout_w, b1_w, b1_b, b2_w, b2_b, tok_w, tok_b,
           dw_bias):
    x = np.asarray(x)
    (w1p, w1lo, dstat, tokw, tokb, w2, dwb, idm, i2s) = _prep_weights(
        ln_w, ln_b, pin_w, pout_w, b1_w, b1_b, b2_w, b2_b, tok_w, tok_b,
        dw_bias)
    if "l1" not in _cache:
        _cache["l1"] = _build_l1()
    if "l2" not in _cache:
        _cache["l2"] = _build_l2()

    xbf = x.astype(BF)
    in_maps1 = []
    for i in range(8):
        b, t0 = i // 4, 2 * (i % 4)
        xhm = np.zeros((4, C, S), BF)
        v4 = np.zeros((4,), np.float32)
        for k in range(4):
            t = t0 - 1 + k
            if 0 <= t < T:
                xhm[k] = xbf[b, t].reshape(C, S)
                v4[k] = 1.0
        in_maps1.append({
            "xh": xhm, "v4b": np.broadcast_to(v4, (128, 4)).copy(),
            "w1p": w1p, "w1lo": w1lo, "dstat": dstat, "tokw": tokw,
            "tokb": tokb, "dwb": dwb, "idm": idm, "i2s": i2s})
    r1 = run_bass_kernel_spmd(_cache["l1"], in_maps1, core_ids=list(range(8)),
                              trace=TRACE)
    PROF["l1"] = r1

    gated = np.zeros((16, 128, S), BF)
    for i in range(8):
        b, t0 = i // 4, 2 * (i % 4)
        gated[b * 8 + t0] = r1.results[i]["g0"]
        gated[b * 8 + t0 + 1] = r1.results[i]["g1"]

    xr = np.ascontiguousarray(x.reshape(16, 64, S)).astype(BF)
    in_maps2 = []
    for i in range(8):
        cs = slice(i * 2048, (i + 1) * 2048)
        in_maps2.append({
            "gh": np.ascontiguousarray(gated[:, :, cs]),
            "xres": np.ascontiguousarray(xr[:, :, cs]),
            "w2": w2})
    r2 = run_bass_kernel_spmd(_cache["l2"], in_maps2, core_ids=list(range(8)),
                              trace=TRACE)
    PROF["l2"] = r2

    out = np.zeros((16, 64, S), np.float32)
    for i in range(8):
        out[:, :, i * 2048 : (i + 1) * 2048] = r2.results[i]["z"]
    return out.reshape(B, T, C, H, W)
